# revision 1
# baseline (speedup 1.0000x reference)
"""AttentionLSTM Trainium2 kernel (8-core SPMD, data-parallel over batch).

Problem: N=256, T=128, D=512, H=1024.
    h0 = c0 = mean(A, (2,3));  per step:
      M = einsum('nh,nhk->nk', h, A2)/sqrt(H); w = softmax(M)
      attn = einsum('nhk,nk->nh', A2, w)
      act = x_t@Wx + h@Wh + attn@Wattn + b -> i,f,o,g -> LSTM update

Per-core design (32 batch rows):
  - All recurrent matmuls in bf16, accumulated in fp32 PSUM, with PE
    column-tiling (tile_position=(0,32q)) so 4 independent M=32 matmuls
    stream concurrently.
  - attn@Wattn is algebraically folded: P[(n,k),:] = A2[n,:,k]@Wattn is
    precomputed once (f32r matmuls); per step act += wBD.T @ P where wBD is
    the block-diagonal softmax weights - attn itself never materializes.
  - M-phase uses the same diag trick: psum_M = hT.T @ A2sb (+ additive
    block-diagonal -1e30 mask via an identity matmul); one Exp activation
    with accum_out yields both exp(M/32) and its row-sum.
  - x@Wx (+b) is precomputed to DRAM as bf16 hi+lo pairs (fp32-accurate),
    entering the per-step accumulation through a one-hot matmul.
  - Recurrent state transposes (h -> hT) via PE transpose-mode.
"""
import math
from contextlib import ExitStack

import numpy as np
import ml_dtypes

import concourse.bass as bass
import concourse.mybir as mybir
import concourse.tile as tile
from concourse.bass import ts
from concourse.bass_utils import run_bass_kernel_spmd
from concourse.vector_clock import ScopedClock

dt = mybir.dt
AF = mybir.ActivationFunctionType
ALU = mybir.AluOpType

N, T, D, H = 256, 128, 512, 1024
NCORES = 8
NL = N // NCORES          # 32 batch rows per core
G = 4 * H                 # 4096 gate columns
NK = NL * 16              # 512 (n,k) pairs
SCALE = 1.0 / math.sqrt(H)


class PatchedTileContext(tile.TileContext):
    """This walrus build allows at most one sem wait per SP TPB_CTRL
    instruction; put the tail waits on single-wait NoOps before the drain."""

    def _drain_and_barrier(self, tick_clock, wait_clock):
        collector = self.nc.sync.nop(nofuse=True, hint="tail_waits")
        wait_clock.add_sem_waits(
            collector.ins, ScopedClock({None: tick_clock.global_clock})
        )
        waits = list(collector.ins.sync_info.on_wait) if collector.ins.sync_info else []
        collector.ins.sync_info = mybir.SyncInfo(on_wait=waits[:1], on_update=[])
        for w in waits[1:]:
            n = self.nc.sync.nop(nofuse=True, hint="tail_waits")
            n.ins.sync_info = mybir.SyncInfo(on_wait=[w], on_update=[])
        self.nc.sync.drain()
        self.nc.all_engine_barrier()
        assert self.sems is not None
        popped = self.nc._tile_sem_poison_stack.pop()
        assert popped is self._sem_poison
        self.nc.clear_and_free_semaphores(list(self.sems.allocated().values()))
        self.nc.all_engine_barrier()


def split_multi_waits(nc):
    """Walrus here rejects >1 sem wait per instruction: move extras onto
    same-engine NoOps inserted just before the instruction."""
    for f in nc.m.functions:
        for bb in f.blocks:
            new_insts = []
            for inst in bb.instructions:
                si = inst.sync_info
                if si is not None and len(si.on_wait) > 1:
                    waits = list(si.on_wait)
                    for w in waits[:-1]:
                        n = mybir.InstNoOp(
                            name=nc.get_next_instruction_name(),
                            engine=inst.engine,
                            ins=[],
                            outs=[],
                            sync_info=mybir.SyncInfo(on_wait=[w], on_update=[]),
                        )
                        new_insts.append(n)
                    inst.sync_info = mybir.SyncInfo(
                        on_wait=[waits[-1]], on_update=list(si.on_update)
                    )
                new_insts.append(inst)
            try:
                bb.instructions[:] = new_insts
            except TypeError:
                bb.instructions = new_insts


def _np_bf16(a):
    return a.astype(ml_dtypes.bfloat16)


def build(t_steps=T, split=True, reps=1, ablate=()):
    nc = bass.Bass("TRN2", target_bir_lowering=False, debug=False, num_devices=NCORES)

    x_d = nc.dram_tensor("x", [NL, T, D], dt.float32, kind="ExternalInput")
    A_d = nc.dram_tensor("A", [NL, H, 16], dt.float32, kind="ExternalInput")
    Wx_d = nc.dram_tensor("Wx", [D, G], dt.float32, kind="ExternalInput")
    Wh_d = nc.dram_tensor("Wh", [H, G], dt.float32, kind="ExternalInput")
    Wattn_d = nc.dram_tensor("Wattn", [H, G], dt.float32, kind="ExternalInput")
    b_d = nc.dram_tensor("b", [1, G], dt.float32, kind="ExternalInput")
    out_d = nc.dram_tensor("out", [NL, T, H], dt.float32, kind="ExternalOutput")
    # last row of each = bf16 hi/lo of the bias b
    xhi_d = nc.dram_tensor("xhi", [NL * T + 1, G], dt.bfloat16, kind="Internal")
    xlo_d = nc.dram_tensor("xlo", [NL * T + 1, G], dt.bfloat16, kind="Internal")

    # ---- inline constants
    mask_np = np.full((NL, NK), -1e30, np.float32)
    for n in range(NL):
        mask_np[n, 16 * n : 16 * n + 16] = 0.0
    mask_c = nc.inline_tensor(_np_bf16(mask_np), name="maskbd")
    e33_np = np.zeros((NL + 1, NL), np.float32)
    e33_np[:NL, :NL] = np.eye(NL)
    e33_np[NL, :] = 1.0
    e33_c = nc.inline_tensor(_np_bf16(e33_np), name="e33")
    id64_c = nc.inline_tensor(np.tile(np.eye(32, dtype=np.float32), (2, 1)), name="id64")
    z128_c = nc.inline_tensor(np.zeros((1, 128), ml_dtypes.bfloat16), name="z128")
    id128_c = nc.inline_tensor(np.eye(128, dtype=np.float32), name="id128")

    with PatchedTileContext(nc) as tc, ExitStack() as stack:
        persist = stack.enter_context(tc.tile_pool(name="persist", bufs=1))
        Psb = [persist.tile([128, G], dt.bfloat16, tag=f"p{m}", name=f"p{m}") for m in range(4)]
        A2bf = [persist.tile([128, NK], dt.bfloat16, tag=f"a2b{j}", name=f"a2b{j}") for j in range(8)]
        hT = persist.tile([128, 256], dt.bfloat16, tag="hT", name="hT")
        cfull = persist.tile([128, 512], dt.float32, tag="cfull", name="cfull")

        # ============ Phase B: A2 layouts, h0T, c0, P = A2^T @ Wattn
        with (
            tc.tile_pool(name="pcb1", bufs=1) as pcb1,
            tc.tile_pool(name="pcb2", bufs=2) as pcb2,
            tc.tile_pool(name="pcbps", bufs=3, space="PSUM") as pcbps,
        ):
            A_hnk = A_d[:, :, :].rearrange("n h k -> h n k")  # [H, NL, 16]
            A2r = [pcb1.tile([128, NK], dt.float32r, tag=f"a2r{j}", name=f"a2r{j}") for j in range(8)]
            h0scr = pcb1.tile([128, 32], dt.float32, tag="h0scr", name="h0scr")
            for j in range(8):
                nc.sync.dma_start(
                    out=A2r[j].rearrange("h (n k) -> h n k", k=16),
                    in_=A_hnk[ts(j, 128), :, :].bitcast(dt.float32r),
                )
                nc.vector.tensor_copy(A2bf[j], A2r[j].bitcast(dt.float32))
                nc.vector.tensor_reduce(
                    h0scr,
                    A2r[j].bitcast(dt.float32).rearrange("h (n k) -> h n k", k=16),
                    mybir.AxisListType.X,
                    ALU.add,
                )
                nc.scalar.mul(hT[:, ts(j, 32)], h0scr, 1.0 / 16.0)
            # c0 stacked into cfull[64:128], 8 h-slices of 128
            for qq in range(8):
                a2n = pcb2.tile([NL, 128 * 16], dt.float32, tag="a2n", name="a2n")
                nc.sync.dma_start(
                    out=a2n.rearrange("n (h k) -> n h k", k=16),
                    in_=A_d[:, ts(qq, 128), :],
                )
                c0scr = pcb2.tile([NL, 128], dt.float32, tag="c0scr", name="c0scr")
                nc.vector.tensor_reduce(
                    c0scr,
                    a2n.rearrange("n (h k) -> n h k", k=16),
                    mybir.AxisListType.X,
                    ALU.add,
                )
                q, r2 = qq // 4, qq % 4
                nc.scalar.mul(
                    cfull[64 + 32 * q : 96 + 32 * q, ts(r2, 128)], c0scr, 1.0 / 16.0
                )
            # P in two Wattn halves; second half added in place (bf16)
            wat = [pcb1.tile([128, G], dt.float32r, tag=f"wat{j}", name=f"wat{j}") for j in range(4)]
            for half in range(2):
                for j in range(4):
                    nc.sync.dma_start(
                        out=wat[j],
                        in_=Wattn_d[ts(4 * half + j, 128), :].bitcast(dt.float32r),
                    )
                for m in range(4):
                    for c in range(8):
                        pps = pcbps.tile([128, 512], dt.float32, tag="pps", name="pps")
                        for j in range(4):
                            nc.tensor.matmul(
                                pps,
                                A2r[4 * half + j][:, ts(m, 128)],
                                wat[j][:, ts(c, 512)],
                                start=(j == 0),
                                stop=(j == 3),
                            )
                        if half == 0:
                            nc.scalar.copy(Psb[m][:, ts(c, 512)], pps)
                        else:
                            nc.vector.tensor_add(
                                Psb[m][:, ts(c, 512)], pps, Psb[m][:, ts(c, 512)]
                            )

        # ============ Phase A: xact = x @ Wx -> DRAM bf16 hi/lo (+ b row)
        with tc.tile_pool(name="bpool", bufs=1) as bpool:
            b_f = bpool.tile([1, G], dt.float32, tag="b_f", name="b_f")
            nc.sync.dma_start(out=b_f, in_=b_d[:, :])
            bhi = bpool.tile([1, G], dt.bfloat16, tag="bhi", name="bhi")
            nc.vector.tensor_copy(bhi, b_f)
            blo = bpool.tile([1, G], dt.bfloat16, tag="blo", name="blo")
            nc.vector.scalar_tensor_tensor(blo, bhi, -1.0, b_f, op0=ALU.mult, op1=ALU.add)
            nc.sync.dma_start(out=xhi_d[NL * T : NL * T + 1, :], in_=bhi)
            nc.sync.dma_start(out=xlo_d[NL * T : NL * T + 1, :], in_=blo)

        with (
            tc.tile_pool(name="pca1", bufs=1) as pca1,
            tc.tile_pool(name="pca", bufs=2) as pca,
            tc.tile_pool(name="pcaps", bufs=3, space="PSUM") as pcaps,
        ):
            id128r = pca1.tile([128, 128], dt.float32r, tag="id128r", name="id128r")
            nc.sync.dma_start(out=id128r, in_=id128_c[:, :].bitcast(dt.float32r))
            Wxsb = [pca1.tile([128, G], dt.float32r, tag=f"wx{k}", name=f"wx{k}") for k in range(4)]
            for k in range(4):
                nc.sync.dma_start(
                    out=Wxsb[k], in_=Wx_d[ts(k, 128), :].bitcast(dt.float32r)
                )

            for i in range(NL):
                xsb = pca.tile([128, D], dt.float32r, tag="xsb", name="xsb")
                nc.sync.dma_start(out=xsb, in_=x_d[i, :, :].bitcast(dt.float32r))
                xt_ps = pcaps.tile([128, 512], dt.float32r, tag="xtps", name="xtps")
                for k in range(4):
                    nc.tensor.transpose(xt_ps[:, ts(k, 128)], xsb[:, ts(k, 128)], id128r)
                xT = pca.tile([128, 512], dt.float32r, tag="xT", name="xT")
                nc.scalar.copy(xT, xt_ps)
                for c in range(8):
                    aps = pcaps.tile([128, 512], dt.float32, tag="acps", name="acps")
                    for k in range(4):
                        nc.tensor.matmul(
                            aps,
                            xT[:, ts(k, 128)],
                            Wxsb[k][:, ts(c, 512)],
                            start=(k == 0),
                            stop=(k == 3),
                        )
                    hi = pca.tile([128, 512], dt.bfloat16, tag="hi", name="hi")
                    nc.scalar.copy(hi, aps)
                    lo = pca.tile([128, 512], dt.bfloat16, tag="lo", name="lo")
                    nc.vector.scalar_tensor_tensor(
                        lo, hi, -1.0, aps, op0=ALU.mult, op1=ALU.add
                    )
                    nc.sync.dma_start(out=xhi_d[ts(i, 128), ts(c, 512)], in_=hi)
                    nc.sync.dma_start(out=xlo_d[ts(i, 128), ts(c, 512)], in_=lo)

        # ============ Phase C: Wh -> bf16 resident
        whpool = stack.enter_context(tc.tile_pool(name="whpool", bufs=1))
        Whsb = [whpool.tile([128, G], dt.bfloat16, tag=f"wh{j}", name=f"wh{j}") for j in range(8)]
        with tc.tile_pool(name="pcc", bufs=2) as pcc:
            for j in range(8):
                stage = pcc.tile([128, G], dt.float32, tag="whstage", name="whstage")
                nc.sync.dma_start(out=stage, in_=Wh_d[ts(j, 128), :])
                nc.vector.tensor_copy(Whsb[j], stage)

        # ============ Recurrent loop
        loopp = stack.enter_context(tc.tile_pool(name="loopp", bufs=1))
        sbIF = loopp.tile([128, 512], dt.float32, tag="sbIF", name="sbIF")
        o_sb = loopp.tile([64, 512], dt.float32, tag="o_sb", name="o_sb")
        th_sb = loopp.tile([64, 512], dt.float32, tag="th_sb", name="th_sb")
        u_sb = loopp.tile([64, 512], dt.float32, tag="u_sb", name="u_sb")
        v_sb = loopp.tile([64, 512], dt.float32, tag="v_sb", name="v_sb")
        h_sb = loopp.tile([32, 1024], dt.float32, tag="h_sb", name="h_sb")
        wexp = loopp.tile([NL, NK], dt.float32, tag="wexp", name="wexp")
        wexpn = loopp.tile([NL, NK], dt.float32, tag="wexpn", name="wexpn")
        s_sb = loopp.tile([NL, 1], dt.float32, tag="s_sb", name="s_sb")
        rs_sb = loopp.tile([NL, 1], dt.float32, tag="rs_sb", name="rs_sb")
        wBD = loopp.tile([128, 128], dt.bfloat16, tag="wBD", name="wBD")
        maskbf = loopp.tile([NL, NK], dt.bfloat16, tag="maskbf", name="maskbf")
        e33 = loopp.tile([NL + 1, NL], dt.bfloat16, tag="e33t", name="e33t")
        id64 = loopp.tile([64, 32], dt.float32, tag="id64t", name="id64t")
        z128 = loopp.tile([1, 128], dt.bfloat16, tag="z128t", name="z128t")
        nc.sync.dma_start(out=z128, in_=z128_c[:, :])
        nc.sync.dma_start(out=maskbf, in_=mask_c[:, :])
        nc.sync.dma_start(out=e33, in_=e33_c[:, :])
        nc.sync.dma_start(out=id64, in_=id64_c[:, :])

        lps = stack.enter_context(tc.tile_pool(name="lps", bufs=1, space="PSUM"))
        xpool = stack.enter_context(tc.tile_pool(name="xpool", bufs=2))
        xhi_nt = xhi_d[0 : NL * T, :].rearrange("(n t) g -> n t g", t=T)
        xlo_nt = xlo_d[0 : NL * T, :].rearrange("(n t) g -> n t g", t=T)

        # chunk -> (psum tensor, block q, gate column slice)
        # psA blocks: (i,0) (i,1) (f,0) (f,1); psB blocks: (g,0) (g,1) (o,0) (o,1)
        chunk_cols = {}
        for q, (gb, half) in enumerate([(0, 0), (0, 1), (H, 0), (H, 1)]):
            chunk_cols[("A", q)] = slice(gb + 512 * half, gb + 512 * half + 512)
        for q, (gb, half) in enumerate([(3 * H, 0), (3 * H, 1), (2 * H, 0), (2 * H, 1)]):
            chunk_cols[("B", q)] = slice(gb + 512 * half, gb + 512 * half + 512)

        rep_ctx = tc.For_i(0, reps, 1) if reps > 1 else None
        if rep_ctx is not None:
            rep_ctx.__enter__()

        def make_xact(t):
            """Prefetch x-act tiles for step t and open psum groups with the
            zeroing + one-hot x-act matmuls (runs in the previous step's
            tail, keeping PE warm and off the critical path)."""
            xhi_t = xpool.tile([NL + 1, G], dt.bfloat16, tag="xhi", name="xhi")
            xlo_t = xpool.tile([NL + 1, G], dt.bfloat16, tag="xlo", name="xlo")
            nc.sync.dma_start(out=xhi_t[0:NL, :], in_=xhi_nt[:, t, :])
            nc.sync.dma_start(out=xhi_t[NL : NL + 1, :], in_=xhi_d[NL * T : NL * T + 1, :])
            nc.sync.dma_start(out=xlo_t[0:NL, :], in_=xlo_nt[:, t, :])
            nc.sync.dma_start(out=xlo_t[NL : NL + 1, :], in_=xlo_d[NL * T : NL * T + 1, :])
            psA = lps.tile([128, 512], dt.float32, tag="psA", name="psA")
            psB = lps.tile([128, 512], dt.float32, tag="psB", name="psB")
            plan = []
            for which, ps in (("A", psA), ("B", psB)):
                for q in range(4):
                    plan.append((ps[ts(q, 32), :], (0, 32 * q), chunk_cols[(which, q)]))
            nc.tensor.matmul(psA, z128, maskbf[0:1, :], start=True, stop=False)
            nc.tensor.matmul(psB, z128, maskbf[0:1, :], start=True, stop=False)
            order = [0, 4, 1, 5, 2, 6, 3, 7]
            for ci in order:
                dst, tp, cs = plan[ci]
                nc.tensor.matmul(dst, e33, xhi_t[:, cs], start=False, stop=False, tile_position=tp)
            for ci in order:
                dst, tp, cs = plan[ci]
                nc.tensor.matmul(dst, e33, xlo_t[:, cs], start=False, stop=False, tile_position=tp)
            return psA, psB, plan

        nc.vector.memset(wBD, 0.0)
        cur = make_xact(0)
        warm_ps = lps.tile([32, 128], dt.float32, tag="psWarm", name="psWarm")

        for t in range(t_steps):
            psA, psB, plan = cur

            # ---- M-phase (col group 0) interleaved with h@Wh on groups 1-3
            psM = lps.tile([NL, NK], dt.float32, tag="psM", name="psM")
            g123 = [1, 5, 2, 6, 3, 7]  # chunks on col groups 1..3
            g0 = [0, 4]
            for j in range(8):
                nc.tensor.matmul(psM, hT[:, ts(j, 32)], A2bf[j], start=(j == 0), stop=False)
                for ci in (g123[2 * (j % 3)], g123[2 * (j % 3) + 1]):
                    dst, tp, cs = plan[ci]
                    nc.tensor.matmul(dst, hT[:, ts(j, 32)], Whsb[j][:, cs], start=False, stop=False, tile_position=tp)
            nc.tensor.matmul(psM, e33[0:NL, :], maskbf, start=False, stop=True)
            # remaining h@Wh rounds: groups 1-3 get 2 of 3 j-passes above; finish all
            done = {(ci, j) for j in range(8) for ci in (g123[2 * (j % 3)], g123[2 * (j % 3) + 1])}
            rest = [(ci, j) for j in range(8) for ci in [0, 4, 1, 5, 2, 6, 3, 7] if (ci, j) not in done]
            # round-robin the remainder by col group to keep streams busy
            rest.sort(key=lambda cj: (cj[1], cj[0]))
            for ci, j in rest:
                dst, tp, cs = plan[ci]
                nc.tensor.matmul(dst, hT[:, ts(j, 32)], Whsb[j][:, cs], start=False, stop=False, tile_position=tp)

            # ---- softmax
            if "softmax" not in ablate:
                nc.scalar.activation(wexp, psM, AF.Exp, scale=SCALE, accum_out=s_sb)
                nc.vector.reciprocal(rs_sb, s_sb)
                nc.vector.tensor_scalar_mul(wexpn, wexp, rs_sb)
                # ---- wBD (PE transposes of normalized weights)
                psWT = lps.tile([128, 128], dt.float32, tag="psWT", name="psWT")
                for m in range(4):
                    nc.tensor.transpose(psWT[:, ts(m, 32)], wexpn[:, ts(m, 128)], id64[0:32, :])
                nc.scalar.copy(wBD, psWT)

            # ---- act matmuls part 2 (attention via P)
            if "attn" not in ablate:
                order = [0, 4, 1, 5, 2, 6, 3, 7]
                for m in range(4):
                    for ci in order:
                        dst, tp, cs = plan[ci]
                        nc.tensor.matmul(dst, wBD[:, ts(m, 32)], Psb[m][:, cs], start=False, stop=False, tile_position=tp)
            nc.tensor.matmul(psA[:, 0:1], z128, maskbf[0:1, 0:1], start=False, stop=True)
            nc.tensor.matmul(psB[:, 0:1], z128, maskbf[0:1, 0:1], start=False, stop=True)

            # ---- gates + state update
            nc.scalar.activation(sbIF, psA, AF.Sigmoid)
            nc.scalar.activation(psB[0:64, :], psB[0:64, :], AF.Tanh)
            nc.scalar.activation(o_sb, psB[64:128, :], AF.Sigmoid)
            nc.vector.tensor_mul(v_sb, sbIF[64:128, :], cfull[64:128, :])
            nc.vector.tensor_mul(u_sb, sbIF[0:64, :], psB[0:64, :])
            # next step's x-act prefetch + psum-open runs in this tail
            if t + 1 < t_steps:
                nxt = make_xact(t + 1)
            else:
                nxt = None
            # keep-warm matmuls pinned to the DVE chain so the PE's HAM
            # activity window never sees a >3.4us idle gap
            if "warm" not in ablate:
                nc.tensor.matmul(warm_ps, id64[0:32, :], v_sb[0:32, 0:128], start=True, stop=True)
            nc.vector.tensor_add(cfull[64:128, :], u_sb, v_sb)
            nc.scalar.activation(th_sb, cfull[64:128, :], AF.Tanh)
            if "warm" not in ablate:
                nc.tensor.matmul(warm_ps, id64[0:32, :], th_sb[0:32, 0:128], start=True, stop=True)
            nc.vector.tensor_mul(h_sb[:, 0:512], o_sb[0:32, :], th_sb[0:32, :])
            nc.vector.tensor_mul(h_sb[:, 512:1024], o_sb[32:64, :], th_sb[32:64, :])
            if "warm" not in ablate:
                nc.tensor.matmul(warm_ps, id64[0:32, :], h_sb[0:32, 0:128], start=True, stop=True)

            nc.sync.dma_start(out=out_d[:, t, :], in_=h_sb)

            if t + 1 < t_steps:
                psHT = lps.tile([128, 256], dt.float32, tag="psHT", name="psHT")
                for j in range(8):
                    nc.tensor.transpose(
                        psHT[:, ts(j, 32)],
                        h_sb[:, ts(j, 128)],
                        id64[0:32, :],
                    )
                nc.scalar.copy(hT, psHT)
            cur = nxt
        if rep_ctx is not None:
            rep_ctx.__exit__(None, None, None)
    if split:
        split_multi_waits(nc)
    return nc


_CACHE = {}


def _get_nc(t_steps):
    if t_steps not in _CACHE:
        _CACHE[t_steps] = build(t_steps)
    return _CACHE[t_steps]


def kernel(x, A, Wx, Wh, Wattn, b, t_steps=T, trace=False):
    x = np.asarray(x, np.float32)
    A = np.asarray(A, np.float32).reshape(N, H, 16)
    Wx = np.ascontiguousarray(np.asarray(Wx, np.float32))
    Wh = np.ascontiguousarray(np.asarray(Wh, np.float32))
    Wattn = np.ascontiguousarray(np.asarray(Wattn, np.float32))
    b = np.asarray(b, np.float32).reshape(1, G)

    nc = _get_nc(t_steps)
    in_maps = []
    for c in range(NCORES):
        sl = slice(NL * c, NL * (c + 1))
        in_maps.append(
            {
                "x": np.ascontiguousarray(x[sl]),
                "A": np.ascontiguousarray(A[sl]),
                "Wx": Wx,
                "Wh": Wh,
                "Wattn": Wattn,
                "b": b,
            }
        )
    res = run_bass_kernel_spmd(nc, in_maps, core_ids=list(range(NCORES)), trace=trace)
    out = np.concatenate([r["out"] for r in res.results], axis=0)
    if trace:
        kernel.last_exec_time_ns = res.exec_time_ns
    return out


kernel.last_exec_time_ns = None



# revision 10
# speedup vs baseline: 1.0094x; 1.0094x over previous
"""AttentionLSTM Trainium2 kernel (8-core SPMD, data-parallel over batch).

Problem: N=256, T=128, D=512, H=1024.
    h0 = c0 = mean(A, (2,3));  per step:
      M = einsum('nh,nhk->nk', h, A2)/sqrt(H); w = softmax(M)
      attn = einsum('nhk,nk->nh', A2, w)
      act = x_t@Wx + h@Wh + attn@Wattn + b -> i,f,o,g -> LSTM update

Per-core design (32 batch rows):
  - All recurrent matmuls in bf16, accumulated in fp32 PSUM, with PE
    column-tiling (tile_position=(0,32q)) so 4 independent M=32 matmuls
    stream concurrently.
  - attn@Wattn is algebraically folded: P[(n,k),:] = A2[n,:,k]@Wattn is
    precomputed once (f32r matmuls); per step act += wBD.T @ P where wBD is
    the block-diagonal softmax weights - attn itself never materializes.
  - M-phase uses the same diag trick: psum_M = hT.T @ A2sb (+ additive
    block-diagonal -1e30 mask via an identity matmul); one Exp activation
    with accum_out yields both exp(M/32) and its row-sum.
  - x@Wx (+b) is precomputed to DRAM as bf16 hi+lo pairs (fp32-accurate),
    entering the per-step accumulation through a one-hot matmul.
  - Recurrent state transposes (h -> hT) via PE transpose-mode.
"""
import math
from contextlib import ExitStack

import numpy as np
import ml_dtypes

import concourse.bass as bass
import concourse.mybir as mybir
import concourse.tile as tile
from concourse.bass import ts
from concourse.bass_utils import run_bass_kernel_spmd
from concourse.vector_clock import ScopedClock

dt = mybir.dt
AF = mybir.ActivationFunctionType
ALU = mybir.AluOpType

N, T, D, H = 256, 128, 512, 1024
NCORES = 8
NL = N // NCORES          # 32 batch rows per core
G = 4 * H                 # 4096 gate columns
NK = NL * 16              # 512 (n,k) pairs
SCALE = 1.0 / math.sqrt(H)


class PatchedTileContext(tile.TileContext):
    """This walrus build allows at most one sem wait per SP TPB_CTRL
    instruction; put the tail waits on single-wait NoOps before the drain."""

    def _drain_and_barrier(self, tick_clock, wait_clock):
        collector = self.nc.sync.nop(nofuse=True, hint="tail_waits")
        wait_clock.add_sem_waits(
            collector.ins, ScopedClock({None: tick_clock.global_clock})
        )
        waits = list(collector.ins.sync_info.on_wait) if collector.ins.sync_info else []
        collector.ins.sync_info = mybir.SyncInfo(on_wait=waits[:1], on_update=[])
        for w in waits[1:]:
            n = self.nc.sync.nop(nofuse=True, hint="tail_waits")
            n.ins.sync_info = mybir.SyncInfo(on_wait=[w], on_update=[])
        self.nc.sync.drain()
        self.nc.all_engine_barrier()
        assert self.sems is not None
        popped = self.nc._tile_sem_poison_stack.pop()
        assert popped is self._sem_poison
        self.nc.clear_and_free_semaphores(list(self.sems.allocated().values()))
        self.nc.all_engine_barrier()


def split_multi_waits(nc):
    """Walrus here rejects >1 sem wait per instruction: move extras onto
    same-engine NoOps inserted just before the instruction."""
    for f in nc.m.functions:
        for bb in f.blocks:
            new_insts = []
            for inst in bb.instructions:
                si = inst.sync_info
                if si is not None and len(si.on_wait) > 1:
                    waits = list(si.on_wait)
                    for w in waits[:-1]:
                        n = mybir.InstNoOp(
                            name=nc.get_next_instruction_name(),
                            engine=inst.engine,
                            ins=[],
                            outs=[],
                            sync_info=mybir.SyncInfo(on_wait=[w], on_update=[]),
                        )
                        new_insts.append(n)
                    inst.sync_info = mybir.SyncInfo(
                        on_wait=[waits[-1]], on_update=list(si.on_update)
                    )
                new_insts.append(inst)
            try:
                bb.instructions[:] = new_insts
            except TypeError:
                bb.instructions = new_insts


def _np_bf16(a):
    return a.astype(ml_dtypes.bfloat16)


def build(t_steps=T, split=True, reps=1, ablate=(), rest_split=32):
    nc = bass.Bass("TRN2", target_bir_lowering=False, debug=False, num_devices=NCORES)

    x_d = nc.dram_tensor("x", [NL, T, D], dt.float32, kind="ExternalInput")
    A_d = nc.dram_tensor("A", [NL, H, 16], dt.float32, kind="ExternalInput")
    Wx_d = nc.dram_tensor("Wx", [D, G], dt.float32, kind="ExternalInput")
    Wh_d = nc.dram_tensor("Wh", [H, G], dt.float32, kind="ExternalInput")
    Wattn_d = nc.dram_tensor("Wattn", [H, G], dt.float32, kind="ExternalInput")
    b_d = nc.dram_tensor("b", [1, G], dt.float32, kind="ExternalInput")
    out_d = nc.dram_tensor("out", [NL, T, H], dt.float32, kind="ExternalOutput")
    # last row of each = bf16 hi/lo of the bias b
    xhi_d = nc.dram_tensor("xhi", [NL * T + 1, G], dt.bfloat16, kind="Internal")
    xlo_d = nc.dram_tensor("xlo", [NL * T + 1, G], dt.bfloat16, kind="Internal")

    # ---- inline constants
    mask_np = np.full((NL, NK), -1e30, np.float32)
    for n in range(NL):
        mask_np[n, 16 * n : 16 * n + 16] = 0.0
    mask_c = nc.inline_tensor(_np_bf16(mask_np), name="maskbd")
    e33_np = np.zeros((NL + 1, NL), np.float32)
    e33_np[:NL, :NL] = np.eye(NL)
    e33_np[NL, :] = 1.0
    e33_c = nc.inline_tensor(_np_bf16(e33_np), name="e33")
    id64_c = nc.inline_tensor(np.tile(np.eye(32, dtype=np.float32), (2, 1)), name="id64")
    z128_c = nc.inline_tensor(np.zeros((1, 128), ml_dtypes.bfloat16), name="z128")
    id128_c = nc.inline_tensor(np.eye(128, dtype=np.float32), name="id128")

    with PatchedTileContext(nc) as tc, ExitStack() as stack:
        persist = stack.enter_context(tc.tile_pool(name="persist", bufs=1))
        Psb = [persist.tile([128, G], dt.bfloat16, tag=f"p{m}", name=f"p{m}") for m in range(4)]
        A2bf = [persist.tile([128, NK], dt.bfloat16, tag=f"a2b{j}", name=f"a2b{j}") for j in range(8)]
        hT = persist.tile([128, 256], dt.bfloat16, tag="hT", name="hT")
        cfull = persist.tile([128, 512], dt.float32, tag="cfull", name="cfull")

        # ============ Phase B: A2 layouts, h0T, c0, P = A2^T @ Wattn
        with (
            tc.tile_pool(name="pcb1", bufs=1) as pcb1,
            tc.tile_pool(name="pcb2", bufs=2) as pcb2,
            tc.tile_pool(name="pcbps", bufs=3, space="PSUM") as pcbps,
        ):
            A_hnk = A_d[:, :, :].rearrange("n h k -> h n k")  # [H, NL, 16]
            A2r = [pcb1.tile([128, NK], dt.float32r, tag=f"a2r{j}", name=f"a2r{j}") for j in range(8)]
            h0scr = pcb1.tile([128, 32], dt.float32, tag="h0scr", name="h0scr")
            for j in range(8):
                nc.sync.dma_start(
                    out=A2r[j].rearrange("h (n k) -> h n k", k=16),
                    in_=A_hnk[ts(j, 128), :, :].bitcast(dt.float32r),
                )
                nc.vector.tensor_copy(A2bf[j], A2r[j].bitcast(dt.float32))
                nc.vector.tensor_reduce(
                    h0scr,
                    A2r[j].bitcast(dt.float32).rearrange("h (n k) -> h n k", k=16),
                    mybir.AxisListType.X,
                    ALU.add,
                )
                nc.scalar.mul(hT[:, ts(j, 32)], h0scr, 1.0 / 16.0)
            # c0 stacked into cfull[64:128], 8 h-slices of 128
            for qq in range(8):
                a2n = pcb2.tile([NL, 128 * 16], dt.float32, tag="a2n", name="a2n")
                nc.sync.dma_start(
                    out=a2n.rearrange("n (h k) -> n h k", k=16),
                    in_=A_d[:, ts(qq, 128), :],
                )
                c0scr = pcb2.tile([NL, 128], dt.float32, tag="c0scr", name="c0scr")
                nc.vector.tensor_reduce(
                    c0scr,
                    a2n.rearrange("n (h k) -> n h k", k=16),
                    mybir.AxisListType.X,
                    ALU.add,
                )
                q, r2 = qq // 4, qq % 4
                nc.scalar.mul(
                    cfull[64 + 32 * q : 96 + 32 * q, ts(r2, 128)], c0scr, 1.0 / 16.0
                )
            # P in two Wattn halves; second half added in place (bf16)
            wat = [pcb1.tile([128, G], dt.float32r, tag=f"wat{j}", name=f"wat{j}") for j in range(4)]
            for half in range(2):
                for j in range(4):
                    nc.sync.dma_start(
                        out=wat[j],
                        in_=Wattn_d[ts(4 * half + j, 128), :].bitcast(dt.float32r),
                    )
                for m in range(4):
                    for c in range(8):
                        pps = pcbps.tile([128, 512], dt.float32, tag="pps", name="pps")
                        for j in range(4):
                            nc.tensor.matmul(
                                pps,
                                A2r[4 * half + j][:, ts(m, 128)],
                                wat[j][:, ts(c, 512)],
                                start=(j == 0),
                                stop=(j == 3),
                            )
                        if half == 0:
                            nc.scalar.copy(Psb[m][:, ts(c, 512)], pps)
                        else:
                            nc.vector.tensor_add(
                                Psb[m][:, ts(c, 512)], pps, Psb[m][:, ts(c, 512)]
                            )

        # ============ Phase A: xact = x @ Wx -> DRAM bf16 hi/lo (+ b row)
        with tc.tile_pool(name="bpool", bufs=1) as bpool:
            b_f = bpool.tile([1, G], dt.float32, tag="b_f", name="b_f")
            nc.sync.dma_start(out=b_f, in_=b_d[:, :])
            bhi = bpool.tile([1, G], dt.bfloat16, tag="bhi", name="bhi")
            nc.vector.tensor_copy(bhi, b_f)
            blo = bpool.tile([1, G], dt.bfloat16, tag="blo", name="blo")
            nc.vector.scalar_tensor_tensor(blo, bhi, -1.0, b_f, op0=ALU.mult, op1=ALU.add)
            nc.sync.dma_start(out=xhi_d[NL * T : NL * T + 1, :], in_=bhi)
            nc.sync.dma_start(out=xlo_d[NL * T : NL * T + 1, :], in_=blo)

        with (
            tc.tile_pool(name="pca1", bufs=1) as pca1,
            tc.tile_pool(name="pca", bufs=2) as pca,
            tc.tile_pool(name="pcaps", bufs=3, space="PSUM") as pcaps,
        ):
            id128r = pca1.tile([128, 128], dt.float32r, tag="id128r", name="id128r")
            nc.sync.dma_start(out=id128r, in_=id128_c[:, :].bitcast(dt.float32r))
            Wxsb = [pca1.tile([128, G], dt.float32r, tag=f"wx{k}", name=f"wx{k}") for k in range(4)]
            for k in range(4):
                nc.sync.dma_start(
                    out=Wxsb[k], in_=Wx_d[ts(k, 128), :].bitcast(dt.float32r)
                )

            for i in range(NL):
                xsb = pca.tile([128, D], dt.float32r, tag="xsb", name="xsb")
                nc.sync.dma_start(out=xsb, in_=x_d[i, :, :].bitcast(dt.float32r))
                xt_ps = pcaps.tile([128, 512], dt.float32r, tag="xtps", name="xtps")
                for k in range(4):
                    nc.tensor.transpose(xt_ps[:, ts(k, 128)], xsb[:, ts(k, 128)], id128r)
                xT = pca.tile([128, 512], dt.float32r, tag="xT", name="xT")
                nc.scalar.copy(xT, xt_ps)
                for c in range(8):
                    aps = pcaps.tile([128, 512], dt.float32, tag="acps", name="acps")
                    for k in range(4):
                        nc.tensor.matmul(
                            aps,
                            xT[:, ts(k, 128)],
                            Wxsb[k][:, ts(c, 512)],
                            start=(k == 0),
                            stop=(k == 3),
                        )
                    hi = pca.tile([128, 512], dt.bfloat16, tag="hi", name="hi")
                    nc.scalar.copy(hi, aps)
                    lo = pca.tile([128, 512], dt.bfloat16, tag="lo", name="lo")
                    nc.vector.scalar_tensor_tensor(
                        lo, hi, -1.0, aps, op0=ALU.mult, op1=ALU.add
                    )
                    nc.sync.dma_start(out=xhi_d[ts(i, 128), ts(c, 512)], in_=hi)
                    nc.sync.dma_start(out=xlo_d[ts(i, 128), ts(c, 512)], in_=lo)

        # ============ Phase C: Wh -> bf16 resident
        whpool = stack.enter_context(tc.tile_pool(name="whpool", bufs=1))
        Whsb = [whpool.tile([128, G], dt.bfloat16, tag=f"wh{j}", name=f"wh{j}") for j in range(8)]
        with tc.tile_pool(name="pcc", bufs=2) as pcc:
            for j in range(8):
                stage = pcc.tile([128, G], dt.float32, tag="whstage", name="whstage")
                nc.sync.dma_start(out=stage, in_=Wh_d[ts(j, 128), :])
                nc.vector.tensor_copy(Whsb[j], stage)

        # ============ Recurrent loop
        loopp = stack.enter_context(tc.tile_pool(name="loopp", bufs=1))
        sbIF = loopp.tile([128, 512], dt.float32, tag="sbIF", name="sbIF")
        o_sb = loopp.tile([64, 512], dt.float32, tag="o_sb", name="o_sb")
        th_sb = loopp.tile([64, 512], dt.float32, tag="th_sb", name="th_sb")
        u_sb = loopp.tile([64, 512], dt.float32, tag="u_sb", name="u_sb")
        v_sb = loopp.tile([64, 512], dt.float32, tag="v_sb", name="v_sb")
        h_sb = loopp.tile([32, 1024], dt.float32, tag="h_sb", name="h_sb")
        wexp = loopp.tile([NL, NK], dt.float32, tag="wexp", name="wexp")
        wexpn = loopp.tile([NL, NK], dt.float32, tag="wexpn", name="wexpn")
        s_sb = loopp.tile([NL, 1], dt.float32, tag="s_sb", name="s_sb")
        rs_sb = loopp.tile([NL, 1], dt.float32, tag="rs_sb", name="rs_sb")
        wBD = loopp.tile([128, 128], dt.bfloat16, tag="wBD", name="wBD")
        maskbf = loopp.tile([NL, NK], dt.bfloat16, tag="maskbf", name="maskbf")
        e33 = loopp.tile([NL + 1, NL], dt.bfloat16, tag="e33t", name="e33t")
        id64 = loopp.tile([64, 32], dt.float32, tag="id64t", name="id64t")
        z128 = loopp.tile([1, 128], dt.bfloat16, tag="z128t", name="z128t")
        nc.sync.dma_start(out=z128, in_=z128_c[:, :])
        nc.sync.dma_start(out=maskbf, in_=mask_c[:, :])
        nc.sync.dma_start(out=e33, in_=e33_c[:, :])
        nc.sync.dma_start(out=id64, in_=id64_c[:, :])

        lps = stack.enter_context(tc.tile_pool(name="lps", bufs=1, space="PSUM"))
        xpool = stack.enter_context(tc.tile_pool(name="xpool", bufs=2))
        xhi_nt = xhi_d[0 : NL * T, :].rearrange("(n t) g -> n t g", t=T)
        xlo_nt = xlo_d[0 : NL * T, :].rearrange("(n t) g -> n t g", t=T)

        # chunk -> (psum tensor, block q, gate column slice)
        # psA blocks: (i,0) (i,1) (f,0) (f,1); psB blocks: (g,0) (g,1) (o,0) (o,1)
        chunk_cols = {}
        for q, (gb, half) in enumerate([(0, 0), (0, 1), (H, 0), (H, 1)]):
            chunk_cols[("A", q)] = slice(gb + 512 * half, gb + 512 * half + 512)
        for q, (gb, half) in enumerate([(3 * H, 0), (3 * H, 1), (2 * H, 0), (2 * H, 1)]):
            chunk_cols[("B", q)] = slice(gb + 512 * half, gb + 512 * half + 512)

        rep_ctx = tc.For_i(0, reps, 1) if reps > 1 else None
        if rep_ctx is not None:
            rep_ctx.__enter__()

        def make_xact(t):
            """Prefetch x-act tiles for step t and open psum groups with the
            zeroing + one-hot x-act matmuls (runs in the previous step's
            tail, keeping PE warm and off the critical path)."""
            xhi_t = xpool.tile([NL + 1, G], dt.bfloat16, tag="xhi", name="xhi")
            xlo_t = xpool.tile([NL + 1, G], dt.bfloat16, tag="xlo", name="xlo")
            nc.sync.dma_start(out=xhi_t[0:NL, :], in_=xhi_nt[:, t, :])
            nc.sync.dma_start(out=xhi_t[NL : NL + 1, :], in_=xhi_d[NL * T : NL * T + 1, :])
            nc.sync.dma_start(out=xlo_t[0:NL, :], in_=xlo_nt[:, t, :])
            nc.sync.dma_start(out=xlo_t[NL : NL + 1, :], in_=xlo_d[NL * T : NL * T + 1, :])
            psA = lps.tile([128, 512], dt.float32, tag="psA", name="psA")
            psB = lps.tile([128, 512], dt.float32, tag="psB", name="psB")
            plan = []
            for which, ps in (("A", psA), ("B", psB)):
                for q in range(4):
                    plan.append((ps[ts(q, 32), :], (0, 32 * q), chunk_cols[(which, q)]))
            nc.tensor.matmul(psA, z128, maskbf[0:1, :], start=True, stop=False)
            nc.tensor.matmul(psB, z128, maskbf[0:1, :], start=True, stop=False)
            order = [0, 4, 1, 5, 2, 6, 3, 7]
            for ci in order:
                dst, tp, cs = plan[ci]
                nc.tensor.matmul(dst, e33, xhi_t[:, cs], start=False, stop=False, tile_position=tp)
            for ci in order:
                dst, tp, cs = plan[ci]
                nc.tensor.matmul(dst, e33, xlo_t[:, cs], start=False, stop=False, tile_position=tp)
            return psA, psB, plan

        nc.vector.memset(wBD, 0.0)
        cur = make_xact(0)
        warm_ps = lps.tile([32, 128], dt.float32, tag="psWarm", name="psWarm")

        for t in range(t_steps):
            psA, psB, plan = cur

            # ---- M-phase (col group 0) interleaved with h@Wh on groups 1-3
            psM = lps.tile([NL, NK], dt.float32, tag="psM", name="psM")
            g123 = [1, 5, 2, 6, 3, 7]  # chunks on col groups 1..3
            g0 = [0, 4]
            for j in range(8):
                nc.tensor.matmul(psM, hT[:, ts(j, 32)], A2bf[j], start=(j == 0), stop=False)
                for ci in (g123[2 * (j % 3)], g123[2 * (j % 3) + 1]):
                    dst, tp, cs = plan[ci]
                    nc.tensor.matmul(dst, hT[:, ts(j, 32)], Whsb[j][:, cs], start=False, stop=False, tile_position=tp)
            nc.tensor.matmul(psM, e33[0:NL, :], maskbf, start=False, stop=True)
            # remaining h@Wh rounds: groups 1-3 get 2 of 3 j-passes above; finish all
            done = {(ci, j) for j in range(8) for ci in (g123[2 * (j % 3)], g123[2 * (j % 3) + 1])}
            rest = [(ci, j) for j in range(8) for ci in [0, 4, 1, 5, 2, 6, 3, 7] if (ci, j) not in done]
            # round-robin the remainder by col group to keep streams busy
            rest.sort(key=lambda cj: (cj[1], cj[0]))
            # only enough pre-transpose rest to cover softmax latency; the tail
            # runs after attn, overlapping the next step's DVE/Scalar chain
            rest_pre, rest_post = rest[:rest_split], rest[rest_split:]
            for ci, j in rest_pre:
                dst, tp, cs = plan[ci]
                nc.tensor.matmul(dst, hT[:, ts(j, 32)], Whsb[j][:, cs], start=False, stop=False, tile_position=tp)

            # ---- softmax
            if "softmax" not in ablate:
                nc.scalar.activation(wexp, psM, AF.Exp, scale=SCALE, accum_out=s_sb)
                nc.vector.reciprocal(rs_sb, s_sb)
                nc.vector.tensor_scalar_mul(wexpn, wexp, rs_sb)
                # ---- wBD (PE transposes of normalized weights)
                psWT = lps.tile([128, 128], dt.float32, tag="psWT", name="psWT")
                for m in range(4):
                    nc.tensor.transpose(psWT[:, ts(m, 32)], wexpn[:, ts(m, 128)], id64[0:32, :])
                nc.scalar.copy(wBD, psWT)

            # ---- act matmuls part 2 (attention via P)
            if "attn" not in ablate:
                order = [0, 4, 1, 5, 2, 6, 3, 7]
                for m in range(4):
                    for ci in order:
                        dst, tp, cs = plan[ci]
                        nc.tensor.matmul(dst, wBD[:, ts(m, 32)], Psb[m][:, cs], start=False, stop=False, tile_position=tp)
            for ci, j in rest_post:
                dst, tp, cs = plan[ci]
                nc.tensor.matmul(dst, hT[:, ts(j, 32)], Whsb[j][:, cs], start=False, stop=False, tile_position=tp)
            nc.tensor.matmul(psA[:, 0:1], z128, maskbf[0:1, 0:1], start=False, stop=True)
            nc.tensor.matmul(psB[:, 0:1], z128, maskbf[0:1, 0:1], start=False, stop=True)

            # ---- gates + state update
            nc.scalar.activation(sbIF, psA, AF.Sigmoid)
            nc.scalar.activation(psB[0:64, :], psB[0:64, :], AF.Tanh)
            nc.scalar.activation(o_sb, psB[64:128, :], AF.Sigmoid)
            nc.vector.tensor_mul(v_sb, sbIF[64:128, :], cfull[64:128, :])
            nc.vector.tensor_mul(u_sb, sbIF[0:64, :], psB[0:64, :])
            # next step's x-act prefetch + psum-open runs in this tail
            if t + 1 < t_steps:
                nxt = make_xact(t + 1)
            else:
                nxt = None
            # keep-warm matmuls pinned to the DVE chain so the PE's HAM
            # activity window never sees a >3.4us idle gap
            if "warm" not in ablate:
                nc.tensor.matmul(warm_ps, id64[0:32, :], v_sb[0:32, 0:128], start=True, stop=True)
            nc.vector.tensor_add(cfull[64:128, :], u_sb, v_sb)
            nc.scalar.activation(th_sb, cfull[64:128, :], AF.Tanh)
            if "warm" not in ablate:
                nc.tensor.matmul(warm_ps, id64[0:32, :], th_sb[0:32, 0:128], start=True, stop=True)
            nc.vector.tensor_mul(h_sb[:, 0:512], o_sb[0:32, :], th_sb[0:32, :])
            nc.vector.tensor_mul(h_sb[:, 512:1024], o_sb[32:64, :], th_sb[32:64, :])
            if "warm" not in ablate:
                nc.tensor.matmul(warm_ps, id64[0:32, :], h_sb[0:32, 0:128], start=True, stop=True)

            nc.sync.dma_start(out=out_d[:, t, :], in_=h_sb)

            if t + 1 < t_steps:
                psHT = lps.tile([128, 256], dt.float32, tag="psHT", name="psHT")
                for j in range(8):
                    nc.tensor.transpose(
                        psHT[:, ts(j, 32)],
                        h_sb[:, ts(j, 128)],
                        id64[0:32, :],
                    )
                nc.scalar.copy(hT, psHT)
            cur = nxt
        if rep_ctx is not None:
            rep_ctx.__exit__(None, None, None)
    if split:
        split_multi_waits(nc)
    return nc


_CACHE = {}


def _get_nc(t_steps):
    if t_steps not in _CACHE:
        _CACHE[t_steps] = build(t_steps)
    return _CACHE[t_steps]


def kernel(x, A, Wx, Wh, Wattn, b, t_steps=T, trace=False):
    x = np.asarray(x, np.float32)
    A = np.asarray(A, np.float32).reshape(N, H, 16)
    Wx = np.ascontiguousarray(np.asarray(Wx, np.float32))
    Wh = np.ascontiguousarray(np.asarray(Wh, np.float32))
    Wattn = np.ascontiguousarray(np.asarray(Wattn, np.float32))
    b = np.asarray(b, np.float32).reshape(1, G)

    nc = _get_nc(t_steps)
    in_maps = []
    for c in range(NCORES):
        sl = slice(NL * c, NL * (c + 1))
        in_maps.append(
            {
                "x": np.ascontiguousarray(x[sl]),
                "A": np.ascontiguousarray(A[sl]),
                "Wx": Wx,
                "Wh": Wh,
                "Wattn": Wattn,
                "b": b,
            }
        )
    res = run_bass_kernel_spmd(nc, in_maps, core_ids=list(range(NCORES)), trace=trace)
    out = np.concatenate([r["out"] for r in res.results], axis=0)
    if trace:
        kernel.last_exec_time_ns = res.exec_time_ns
    return out


kernel.last_exec_time_ns = None



# revision 15
# speedup vs baseline: 1.0493x; 1.0395x over previous
"""AttentionLSTM Trainium2 kernel (8-core SPMD, data-parallel over batch).

Problem: N=256, T=128, D=512, H=1024.
    h0 = c0 = mean(A, (2,3));  per step:
      M = einsum('nh,nhk->nk', h, A2)/sqrt(H); w = softmax(M)
      attn = einsum('nhk,nk->nh', A2, w)
      act = x_t@Wx + h@Wh + attn@Wattn + b -> i,f,o,g -> LSTM update

Per-core design (32 batch rows):
  - All recurrent matmuls in bf16, accumulated in fp32 PSUM, with PE
    column-tiling (tile_position=(0,32q)) so 4 independent M=32 matmuls
    stream concurrently.
  - attn@Wattn is algebraically folded: P[(n,k),:] = A2[n,:,k]@Wattn is
    precomputed once (f32r matmuls); per step act += wBD.T @ P where wBD is
    the block-diagonal softmax weights - attn itself never materializes.
  - M-phase uses the same diag trick: psum_M = hT.T @ A2sb (+ additive
    block-diagonal -1e30 mask via an identity matmul); one Exp activation
    with accum_out yields both exp(M/32) and its row-sum.
  - x@Wx (+b) is precomputed to DRAM as bf16 hi+lo pairs (fp32-accurate),
    entering the per-step accumulation through a one-hot matmul.
  - Recurrent state transposes (h -> hT) via PE transpose-mode.
"""
import math
from contextlib import ExitStack

import numpy as np
import ml_dtypes

import concourse.bass as bass
import concourse.mybir as mybir
import concourse.tile as tile
from concourse.bass import ts
from concourse.bass_utils import run_bass_kernel_spmd
from concourse.vector_clock import ScopedClock

dt = mybir.dt
AF = mybir.ActivationFunctionType
ALU = mybir.AluOpType

N, T, D, H = 256, 128, 512, 1024
NCORES = 8
NL = N // NCORES          # 32 batch rows per core
G = 4 * H                 # 4096 gate columns
NK = NL * 16              # 512 (n,k) pairs
SCALE = 1.0 / math.sqrt(H)


class PatchedTileContext(tile.TileContext):
    """This walrus build allows at most one sem wait per SP TPB_CTRL
    instruction; put the tail waits on single-wait NoOps before the drain."""

    def _drain_and_barrier(self, tick_clock, wait_clock):
        collector = self.nc.sync.nop(nofuse=True, hint="tail_waits")
        wait_clock.add_sem_waits(
            collector.ins, ScopedClock({None: tick_clock.global_clock})
        )
        waits = list(collector.ins.sync_info.on_wait) if collector.ins.sync_info else []
        collector.ins.sync_info = mybir.SyncInfo(on_wait=waits[:1], on_update=[])
        for w in waits[1:]:
            n = self.nc.sync.nop(nofuse=True, hint="tail_waits")
            n.ins.sync_info = mybir.SyncInfo(on_wait=[w], on_update=[])
        self.nc.sync.drain()
        self.nc.all_engine_barrier()
        assert self.sems is not None
        popped = self.nc._tile_sem_poison_stack.pop()
        assert popped is self._sem_poison
        self.nc.clear_and_free_semaphores(list(self.sems.allocated().values()))
        self.nc.all_engine_barrier()


def split_multi_waits(nc):
    """Walrus here rejects >1 sem wait per instruction: move extras onto
    same-engine NoOps inserted just before the instruction."""
    for f in nc.m.functions:
        for bb in f.blocks:
            new_insts = []
            for inst in bb.instructions:
                si = inst.sync_info
                if si is not None and len(si.on_wait) > 1:
                    waits = list(si.on_wait)
                    for w in waits[:-1]:
                        n = mybir.InstNoOp(
                            name=nc.get_next_instruction_name(),
                            engine=inst.engine,
                            ins=[],
                            outs=[],
                            sync_info=mybir.SyncInfo(on_wait=[w], on_update=[]),
                        )
                        new_insts.append(n)
                    inst.sync_info = mybir.SyncInfo(
                        on_wait=[waits[-1]], on_update=list(si.on_update)
                    )
                new_insts.append(inst)
            try:
                bb.instructions[:] = new_insts
            except TypeError:
                bb.instructions = new_insts


def _np_bf16(a):
    return a.astype(ml_dtypes.bfloat16)


def build(t_steps=T, split=True, reps=1, ablate=(), rest_split=32):
    nc = bass.Bass("TRN2", target_bir_lowering=False, debug=False, num_devices=NCORES)

    x_d = nc.dram_tensor("x", [NL, T, D], dt.float32, kind="ExternalInput")
    A_d = nc.dram_tensor("A", [NL, H, 16], dt.float32, kind="ExternalInput")
    Wx_d = nc.dram_tensor("Wx", [D, G], dt.float32, kind="ExternalInput")
    Wh_d = nc.dram_tensor("Wh", [H, G], dt.float32, kind="ExternalInput")
    Wattn_d = nc.dram_tensor("Wattn", [H, G], dt.float32, kind="ExternalInput")
    b_d = nc.dram_tensor("b", [1, G], dt.float32, kind="ExternalInput")
    out_d = nc.dram_tensor("out", [NL, T, H], dt.float32, kind="ExternalOutput")

    # ---- inline constants
    mask_np = np.full((NL, NK), -1e30, np.float32)
    for n in range(NL):
        mask_np[n, 16 * n : 16 * n + 16] = 0.0
    mask_c = nc.inline_tensor(_np_bf16(mask_np), name="maskbd")
    e33_np = np.zeros((NL + 1, NL), np.float32)
    e33_np[:NL, :NL] = np.eye(NL)
    e33_np[NL, :] = 1.0
    e33_c = nc.inline_tensor(_np_bf16(e33_np), name="e33")
    id64_c = nc.inline_tensor(np.tile(np.eye(32, dtype=np.float32), (2, 1)), name="id64")
    z128_c = nc.inline_tensor(np.zeros((1, 128), ml_dtypes.bfloat16), name="z128")
    ones1_c = nc.inline_tensor(np.ones((1, 32), ml_dtypes.bfloat16), name="ones1")

    with PatchedTileContext(nc) as tc, ExitStack() as stack:
        persist = stack.enter_context(tc.tile_pool(name="persist", bufs=1))
        Psb = [persist.tile([128, G], dt.bfloat16, tag=f"p{m}", name=f"p{m}") for m in range(4)]
        A2bf = [persist.tile([128, NK], dt.bfloat16, tag=f"a2b{j}", name=f"a2b{j}") for j in range(8)]
        hT = persist.tile([128, 256], dt.bfloat16, tag="hT", name="hT")
        cfull = persist.tile([128, 512], dt.float32, tag="cfull", name="cfull")

        # ============ Phase B: A2 layouts, h0T, c0, P = A2^T @ Wattn
        with (
            tc.tile_pool(name="pcb1", bufs=1) as pcb1,
            tc.tile_pool(name="pcb2", bufs=2) as pcb2,
            tc.tile_pool(name="pcbps", bufs=3, space="PSUM") as pcbps,
        ):
            A_hnk = A_d[:, :, :].rearrange("n h k -> h n k")  # [H, NL, 16]
            A2r = [pcb1.tile([128, NK], dt.float32r, tag=f"a2r{j}", name=f"a2r{j}") for j in range(8)]
            h0scr = pcb1.tile([128, 32], dt.float32, tag="h0scr", name="h0scr")
            for j in range(8):
                nc.sync.dma_start(
                    out=A2r[j].rearrange("h (n k) -> h n k", k=16),
                    in_=A_hnk[ts(j, 128), :, :].bitcast(dt.float32r),
                )
                nc.vector.tensor_copy(A2bf[j], A2r[j].bitcast(dt.float32))
                nc.vector.tensor_reduce(
                    h0scr,
                    A2r[j].bitcast(dt.float32).rearrange("h (n k) -> h n k", k=16),
                    mybir.AxisListType.X,
                    ALU.add,
                )
                nc.scalar.mul(hT[:, ts(j, 32)], h0scr, 1.0 / 16.0)
            # c0 stacked into cfull[64:128], 8 h-slices of 128
            for qq in range(8):
                a2n = pcb2.tile([NL, 128 * 16], dt.float32, tag="a2n", name="a2n")
                nc.sync.dma_start(
                    out=a2n.rearrange("n (h k) -> n h k", k=16),
                    in_=A_d[:, ts(qq, 128), :],
                )
                c0scr = pcb2.tile([NL, 128], dt.float32, tag="c0scr", name="c0scr")
                nc.vector.tensor_reduce(
                    c0scr,
                    a2n.rearrange("n (h k) -> n h k", k=16),
                    mybir.AxisListType.X,
                    ALU.add,
                )
                q, r2 = qq // 4, qq % 4
                nc.scalar.mul(
                    cfull[64 + 32 * q : 96 + 32 * q, ts(r2, 128)], c0scr, 1.0 / 16.0
                )
            # P in two Wattn halves; second half added in place (bf16)
            wat = [pcb1.tile([128, G], dt.float32r, tag=f"wat{j}", name=f"wat{j}") for j in range(4)]
            for half in range(2):
                for j in range(4):
                    nc.sync.dma_start(
                        out=wat[j],
                        in_=Wattn_d[ts(4 * half + j, 128), :].bitcast(dt.float32r),
                    )
                for m in range(4):
                    for c in range(8):
                        pps = pcbps.tile([128, 512], dt.float32, tag="pps", name="pps")
                        for j in range(4):
                            nc.tensor.matmul(
                                pps,
                                A2r[4 * half + j][:, ts(m, 128)],
                                wat[j][:, ts(c, 512)],
                                start=(j == 0),
                                stop=(j == 3),
                            )
                        if half == 0:
                            nc.scalar.copy(Psb[m][:, ts(c, 512)], pps)
                        else:
                            nc.vector.tensor_add(
                                Psb[m][:, ts(c, 512)], pps, Psb[m][:, ts(c, 512)]
                            )

        # ============ Phase A': resident Wx (bf16) + bias row (bf16)
        wxpool = stack.enter_context(tc.tile_pool(name="wxpool", bufs=1))
        Wxsb = [wxpool.tile([128, G], dt.bfloat16, tag=f"wx{k}", name=f"wx{k}") for k in range(4)]
        bbf = wxpool.tile([1, G], dt.bfloat16, tag="bbf", name="bbf")
        ones1 = wxpool.tile([1, 32], dt.bfloat16, tag="ones1", name="ones1")
        nc.sync.dma_start(out=ones1, in_=ones1_c[:, :])
        with tc.tile_pool(name="bpool", bufs=2) as bpool:
            b_f = bpool.tile([1, G], dt.float32, tag="b_f", name="b_f")
            nc.sync.dma_start(out=b_f, in_=b_d[:, :])
            nc.vector.tensor_copy(bbf, b_f)
            for k in range(4):
                for hh in range(2):
                    wst = bpool.tile([128, G // 2], dt.float32, tag="wxstage", name="wxstage")
                    nc.sync.dma_start(
                        out=wst, in_=Wx_d[ts(k, 128), 2048 * hh : 2048 * hh + 2048]
                    )
                    nc.vector.tensor_copy(
                        Wxsb[k][:, 2048 * hh : 2048 * hh + 2048], wst
                    )

        # ============ Phase C: Wh -> bf16 resident
        whpool = stack.enter_context(tc.tile_pool(name="whpool", bufs=1))
        Whsb = [whpool.tile([128, G], dt.bfloat16, tag=f"wh{j}", name=f"wh{j}") for j in range(8)]
        with tc.tile_pool(name="pcc", bufs=2) as pcc:
            for j in range(8):
                for hh in range(2):
                    stage = pcc.tile([128, G // 2], dt.float32, tag="whstage", name="whstage")
                    nc.sync.dma_start(
                        out=stage, in_=Wh_d[ts(j, 128), 2048 * hh : 2048 * hh + 2048]
                    )
                    nc.vector.tensor_copy(
                        Whsb[j][:, 2048 * hh : 2048 * hh + 2048], stage
                    )

        # ============ Recurrent loop
        loopp = stack.enter_context(tc.tile_pool(name="loopp", bufs=1))
        sbIF = loopp.tile([128, 512], dt.float32, tag="sbIF", name="sbIF")
        o_sb = loopp.tile([64, 512], dt.float32, tag="o_sb", name="o_sb")
        th_sb = loopp.tile([64, 512], dt.float32, tag="th_sb", name="th_sb")
        u_sb = loopp.tile([64, 512], dt.float32, tag="u_sb", name="u_sb")
        v_sb = loopp.tile([64, 512], dt.float32, tag="v_sb", name="v_sb")
        h_sb = loopp.tile([32, 1024], dt.float32, tag="h_sb", name="h_sb")
        wexp = loopp.tile([NL, NK], dt.float32, tag="wexp", name="wexp")
        wexpn = loopp.tile([NL, NK], dt.float32, tag="wexpn", name="wexpn")
        s_sb = loopp.tile([NL, 1], dt.float32, tag="s_sb", name="s_sb")
        rs_sb = loopp.tile([NL, 1], dt.float32, tag="rs_sb", name="rs_sb")
        wBD = loopp.tile([128, 128], dt.bfloat16, tag="wBD", name="wBD")
        maskbf = loopp.tile([NL, NK], dt.bfloat16, tag="maskbf", name="maskbf")
        e33 = loopp.tile([NL + 1, NL], dt.bfloat16, tag="e33t", name="e33t")
        id64 = loopp.tile([64, 32], dt.float32, tag="id64t", name="id64t")
        z128 = loopp.tile([1, 128], dt.bfloat16, tag="z128t", name="z128t")
        nc.sync.dma_start(out=z128, in_=z128_c[:, :])
        nc.sync.dma_start(out=maskbf, in_=mask_c[:, :])
        nc.sync.dma_start(out=e33, in_=e33_c[:, :])
        nc.sync.dma_start(out=id64, in_=id64_c[:, :])

        lps = stack.enter_context(tc.tile_pool(name="lps", bufs=1, space="PSUM"))
        lps2 = stack.enter_context(tc.tile_pool(name="lps2", bufs=2, space="PSUM"))
        xq = stack.enter_context(tc.tile_pool(name="xq", bufs=2))
        xTp = stack.enter_context(tc.tile_pool(name="xTp", bufs=2))

        # chunk -> (psum tensor, block q, gate column slice)
        # psA blocks: (i,0) (i,1) (f,0) (f,1); psB blocks: (g,0) (g,1) (o,0) (o,1)
        chunk_cols = {}
        for q, (gb, half) in enumerate([(0, 0), (0, 1), (H, 0), (H, 1)]):
            chunk_cols[("A", q)] = slice(gb + 512 * half, gb + 512 * half + 512)
        for q, (gb, half) in enumerate([(3 * H, 0), (3 * H, 1), (2 * H, 0), (2 * H, 1)]):
            chunk_cols[("B", q)] = slice(gb + 512 * half, gb + 512 * half + 512)

        rep_ctx = tc.For_i(0, reps, 1) if reps > 1 else None
        if rep_ctx is not None:
            rep_ctx.__enter__()

        def load_x(t):
            """DMA x[:, t, :] -> [NL, D] f32 staging tile."""
            xsb = xq.tile([NL, D], dt.float32, tag="xsb", name="xsb")
            nc.sync.dma_start(out=xsb, in_=x_d[:, t, :])
            return xsb

        def prep_xT(xsb):
            """PE-transpose x_t [NL, D] -> xT hi/lo bf16 [128, 128]
            (4 d-chunks of 32 batch cols each), via PSUM."""
            psXT = lps.tile([128, 128], dt.float32, tag="psXT", name="psXT")
            for k in range(4):
                nc.tensor.transpose(psXT[:, ts(k, 32)], xsb[:, ts(k, 128)], id64[0:32, :])
            xThi = xTp.tile([128, 128], dt.bfloat16, tag="xThi", name="xThi")
            nc.scalar.copy(xThi, psXT)
            xTlo = xTp.tile([128, 128], dt.bfloat16, tag="xTlo", name="xTlo")
            nc.vector.scalar_tensor_tensor(
                xTlo, xThi, -1.0, psXT, op0=ALU.mult, op1=ALU.add
            )
            return xThi, xTlo

        def make_xact_ps(xT):
            """Open this step's psA/psB with x_t @ Wx (bf16 hi/lo x) + bias."""
            xThi, xTlo = xT
            psA = lps2.tile([128, 512], dt.float32, tag="psA", name="psA")
            psB = lps2.tile([128, 512], dt.float32, tag="psB", name="psB")
            plan = []
            for which, ps in (("A", psA), ("B", psB)):
                for q in range(4):
                    plan.append((ps[ts(q, 32), :], (0, 32 * q), chunk_cols[(which, q)]))
            order = [0, 4, 1, 5, 2, 6, 3, 7]
            for k in range(4):
                for ci in order:
                    dst, tp, cs = plan[ci]
                    nc.tensor.matmul(dst, xThi[:, ts(k, 32)], Wxsb[k][:, cs], start=(k == 0), stop=False, tile_position=tp)
            for k in range(4):
                for ci in order:
                    dst, tp, cs = plan[ci]
                    nc.tensor.matmul(dst, xTlo[:, ts(k, 32)], Wxsb[k][:, cs], start=False, stop=False, tile_position=tp)
            for ci in order:
                dst, tp, cs = plan[ci]
                nc.tensor.matmul(dst, ones1, bbf[0:1, cs], start=False, stop=False, tile_position=tp)
            return psA, psB, plan

        nc.vector.memset(wBD, 0.0)
        xsb_q = [load_x(0)]
        if t_steps > 1:
            xsb_q.append(load_x(1))
        xT_q = [prep_xT(x) for x in xsb_q]
        cur = make_xact_ps(xT_q[0])

        for t in range(t_steps):
            psA, psB, plan = cur
            if t + 2 < t_steps:
                xsb2 = load_x(t + 2)

            # ---- M-phase (col group 0) interleaved with h@Wh on groups 1-3
            psM = lps.tile([NL, NK], dt.float32, tag="psM", name="psM")
            g123 = [1, 5, 2, 6, 3, 7]  # chunks on col groups 1..3
            g0 = [0, 4]
            for j in range(8):
                nc.tensor.matmul(psM, hT[:, ts(j, 32)], A2bf[j], start=(j == 0), stop=False)
                for ci in (g123[2 * (j % 3)], g123[2 * (j % 3) + 1]):
                    dst, tp, cs = plan[ci]
                    nc.tensor.matmul(dst, hT[:, ts(j, 32)], Whsb[j][:, cs], start=False, stop=False, tile_position=tp)
            nc.tensor.matmul(psM, e33[0:NL, :], maskbf, start=False, stop=True)
            # remaining h@Wh rounds: groups 1-3 get 2 of 3 j-passes above; finish all
            done = {(ci, j) for j in range(8) for ci in (g123[2 * (j % 3)], g123[2 * (j % 3) + 1])}
            rest = [(ci, j) for j in range(8) for ci in [0, 4, 1, 5, 2, 6, 3, 7] if (ci, j) not in done]
            # round-robin the remainder by col group to keep streams busy
            rest.sort(key=lambda cj: (cj[1], cj[0]))
            # only enough pre-transpose rest to cover softmax latency; the tail
            # runs after attn, overlapping the next step's DVE/Scalar chain
            rest_pre, rest_post = rest[:rest_split], rest[rest_split:]
            for ci, j in rest_pre:
                dst, tp, cs = plan[ci]
                nc.tensor.matmul(dst, hT[:, ts(j, 32)], Whsb[j][:, cs], start=False, stop=False, tile_position=tp)

            # ---- softmax
            if "softmax" not in ablate:
                nc.scalar.activation(wexp, psM, AF.Exp, scale=SCALE, accum_out=s_sb)
                nc.vector.reciprocal(rs_sb, s_sb)
                nc.vector.tensor_scalar_mul(wexpn, wexp, rs_sb)
                # ---- wBD (PE transposes of normalized weights)
                psWT = lps.tile([128, 128], dt.float32, tag="psWT", name="psWT")
                for m in range(4):
                    nc.tensor.transpose(psWT[:, ts(m, 32)], wexpn[:, ts(m, 128)], id64[0:32, :])
                nc.scalar.copy(wBD, psWT)

            # ---- act matmuls part 2 (attention via P)
            if "attn" not in ablate:
                order = [0, 4, 1, 5, 2, 6, 3, 7]
                for m in range(4):
                    for ci in order:
                        dst, tp, cs = plan[ci]
                        nc.tensor.matmul(dst, wBD[:, ts(m, 32)], Psb[m][:, cs], start=False, stop=False, tile_position=tp)
            for ci, j in rest_post:
                dst, tp, cs = plan[ci]
                nc.tensor.matmul(dst, hT[:, ts(j, 32)], Whsb[j][:, cs], start=False, stop=False, tile_position=tp)
            nc.tensor.matmul(psA[:, 0:1], z128, maskbf[0:1, 0:1], start=False, stop=True)
            nc.tensor.matmul(psB[:, 0:1], z128, maskbf[0:1, 0:1], start=False, stop=True)

            # ---- gates + state update
            nc.scalar.activation(sbIF, psA, AF.Sigmoid)
            nc.scalar.activation(psB[0:64, :], psB[0:64, :], AF.Tanh)
            nc.scalar.activation(o_sb, psB[64:128, :], AF.Sigmoid)
            nc.vector.tensor_mul(v_sb, sbIF[64:128, :], cfull[64:128, :])
            nc.vector.tensor_mul(u_sb, sbIF[0:64, :], psB[0:64, :])
            # next step's x-act matmuls + t+2 transpose run in this tail,
            # filling the PE during the DVE/Scalar gate chain
            if t + 1 < t_steps:
                nxt = make_xact_ps(xT_q[1])
            else:
                nxt = None
            if t + 2 < t_steps:
                xT_q = [xT_q[1], prep_xT(xsb2)]
            nc.vector.tensor_add(cfull[64:128, :], u_sb, v_sb)
            nc.scalar.activation(th_sb, cfull[64:128, :], AF.Tanh)
            nc.vector.tensor_mul(h_sb[:, 0:512], o_sb[0:32, :], th_sb[0:32, :])
            nc.vector.tensor_mul(h_sb[:, 512:1024], o_sb[32:64, :], th_sb[32:64, :])

            nc.sync.dma_start(out=out_d[:, t, :], in_=h_sb)

            if t + 1 < t_steps:
                psHT = lps.tile([128, 256], dt.float32, tag="psHT", name="psHT")
                for j in range(8):
                    nc.tensor.transpose(
                        psHT[:, ts(j, 32)],
                        h_sb[:, ts(j, 128)],
                        id64[0:32, :],
                    )
                nc.scalar.copy(hT, psHT)
            cur = nxt
        if rep_ctx is not None:
            rep_ctx.__exit__(None, None, None)
    if split:
        split_multi_waits(nc)
    return nc


_CACHE = {}


def _get_nc(t_steps):
    if t_steps not in _CACHE:
        _CACHE[t_steps] = build(t_steps)
    return _CACHE[t_steps]


def kernel(x, A, Wx, Wh, Wattn, b, t_steps=T, trace=False):
    x = np.asarray(x, np.float32)
    A = np.asarray(A, np.float32).reshape(N, H, 16)
    Wx = np.ascontiguousarray(np.asarray(Wx, np.float32))
    Wh = np.ascontiguousarray(np.asarray(Wh, np.float32))
    Wattn = np.ascontiguousarray(np.asarray(Wattn, np.float32))
    b = np.asarray(b, np.float32).reshape(1, G)

    nc = _get_nc(t_steps)
    in_maps = []
    for c in range(NCORES):
        sl = slice(NL * c, NL * (c + 1))
        in_maps.append(
            {
                "x": np.ascontiguousarray(x[sl]),
                "A": np.ascontiguousarray(A[sl]),
                "Wx": Wx,
                "Wh": Wh,
                "Wattn": Wattn,
                "b": b,
            }
        )
    res = run_bass_kernel_spmd(nc, in_maps, core_ids=list(range(NCORES)), trace=trace)
    out = np.concatenate([r["out"] for r in res.results], axis=0)
    if trace:
        kernel.last_exec_time_ns = res.exec_time_ns
    return out


kernel.last_exec_time_ns = None

